# revision 19
# baseline (speedup 1.0000x reference)
"""Trainium2 Bass kernel for nn_Block_68822555951557 (differential cross-attention
transformer block + MLP).

Sharding: 8 cores; core c handles batch b=c//4 and query rows [(c%4)*512, +512).
Each core is fully independent (no collectives): it recomputes K/V for its batch
from ocean, runs all 8 heads for its 512 query rows, then Wo/LN/MLP on its rows.

Compute layout notes:
- Weights are pre-transposed on host and passed as extra inputs.
- Activation transposes (x, ocean, out, y, h) are done on-device with the PE.
- Attention scores are computed in S^T layout: psum[k=128, q=512] via
  lhsT = K^T d-slice (32,128), rhs = Q^T d-slice (32,512), float32r (1 cyc/row).
- exp on ScalarE with fused *SCALE, bf16 output; softmax denominators come for
  free from an appended ones-column in V' during the U@V' matmul.
- att@V: out[q,33] accumulated over 16 k-tiles into one packed PSUM bank per
  head: cols [66q+0:33]=U1@[V|1], [66q+33:66]=U2@[lam*V|1] for each q-tile.
- Combine per (head, qtile): out = U1V*r1 - lamU2V*r2, r=1/s from ones cols.
"""

import sys

if "/opt/trn_rl_repo" not in sys.path:
    sys.path.insert(0, "/opt/trn_rl_repo")

import numpy as np

import concourse.bass as bass
import concourse.mybir as mybir
import concourse.tile as tile
from concourse import bacc
from concourse.bass import ds, ts
from concourse.bass_utils import run_bass_kernel_spmd
from concourse.masks import make_identity

F32 = mybir.dt.float32
F32R = mybir.dt.float32r
BF16 = mybir.dt.bfloat16
AF = mybir.ActivationFunctionType
OP = mybir.AluOpType

DIM = 256
H = 8
HD = 32
EPS = 1e-5
LAMBDA_INIT = 0.1
SCALE = HD ** -0.5
B, Nd, No = 2, 2048, 2048
NCORES = 8
QR = (B * Nd) // NCORES  # 512 query rows per core
QT_N = QR // 128  # 4 q tiles
KT_N = No // 128  # 16 k tiles

# dtype for the U (=exp(S)) and V' operands of the att@V matmul.
UV_DT = F32
# ATT_V2: att@V with V' stationary producing out^T directly (f32r, 1cyc/row,
# 4x fewer matmuls and no per-matmul 128x128 weight reload).
ATT_V2 = True




def _layernorm(nc, small, out_tiles, in_tiles, gamma_bc, beta_bc):
    """LayerNorm along free axis (256) for a list of (128,256) tiles."""
    for o, t in zip(out_tiles, in_tiles):
        stats = small.tile([128, 6], F32, tag="ln_stats", name="ln_stats")
        mv = small.tile([128, 2], F32, tag="ln_mv", name="ln_mv")
        nc.vector.bn_stats(out=stats[:], in_=t[:])
        nc.vector.bn_aggr(out=mv[:], in_=stats[:])
        eps_t = small.tile([128, 1], F32, tag="ln_eps", name="ln_eps")
        nc.vector.memset(eps_t[:], EPS)
        rstd = small.tile([128, 1], F32, tag="ln_rstd", name="ln_rstd")
        nc.scalar.activation(
            out=rstd[:], in_=mv[:, 1:2], func=AF.Sqrt, bias=eps_t[:], scale=1.0
        )
        nc.vector.reciprocal(out=rstd[:], in_=rstd[:])
        nc.vector.tensor_scalar(
            out=o[:],
            in0=t[:],
            scalar1=mv[:, 0:1],
            scalar2=rstd[:],
            op0=OP.subtract,
            op1=OP.mult,
        )
        nc.vector.tensor_mul(out=o[:], in0=o[:], in1=gamma_bc[:])
        nc.vector.tensor_add(out=o[:], in0=o[:], in1=beta_bc[:])


def build_program(nc, iters=1, skip=()):
    """Build the SPMD per-core program (tensors declared on nc).
    iters>1 repeats the body, reusing all buffers (for timing via slope)."""
    # ---- DRAM I/O ----
    d_drift = nc.dram_tensor("drift_s", (QR, DIM), F32, kind="ExternalInput")
    d_ocean = nc.dram_tensor("ocean_b", (No, DIM), F32, kind="ExternalInput")
    d_wqT = nc.dram_tensor("WqT", (DIM, 2 * DIM), F32R, kind="ExternalInput")
    d_wkT = nc.dram_tensor("WkT", (DIM, 2 * DIM), F32R, kind="ExternalInput")
    d_wvT = nc.dram_tensor("WvT", (DIM, DIM), F32R, kind="ExternalInput")
    d_woT = nc.dram_tensor("WoT", (DIM, DIM), F32R, kind="ExternalInput")
    d_fc1T = nc.dram_tensor("fc1T", (DIM, 4 * DIM), F32R, kind="ExternalInput")
    d_fc2T = nc.dram_tensor("fc2T", (4 * DIM, DIM), F32R, kind="ExternalInput")
    d_gamma = nc.dram_tensor("gamma", (DIM,), F32, kind="ExternalInput")
    d_beta = nc.dram_tensor("beta", (DIM,), F32, kind="ExternalInput")
    d_fc1b = nc.dram_tensor("fc1_b", (4 * DIM,), F32, kind="ExternalInput")
    d_fc2b = nc.dram_tensor("fc2_b", (DIM,), F32, kind="ExternalInput")
    d_lam = nc.dram_tensor("lam_rep", (128, DIM), F32, kind="ExternalInput")
    d_out = nc.dram_tensor("dout", (QR, DIM), F32, kind="ExternalOutput")

    def bcast_ap(d1, p=128):
        """DRAM 1-D tensor -> (p, n) AP with partition step 0 (row broadcast)."""
        a = d1.ap()
        return bass.AP(tensor=a.tensor, offset=a.offset, ap=[[0, p], a.ap[0]])

    # Q/K head-dim tiling: Qdim (512) split into 96-row projection tiles so
    # every 32-row (head, half) slice starts at partition 0/32/64 (HW limit).
    # block n = 2h+A lives in tile n//3 at offset 32*(n%3).
    MT_W = [96, 96, 96, 96, 96, 32]
    MT_O = [0, 96, 192, 288, 384, 480]

    with tile.TileContext(nc) as tc:
        with (
            tc.tile_pool(name="const", bufs=1) as const,
            tc.tile_pool(name="work", bufs=2) as work,
            tc.tile_pool(name="small", bufs=4) as small,
            tc.tile_pool(name="med", bufs=1) as med,
            tc.tile_pool(name="upool", bufs=2) as upool,
            tc.tile_pool(name="psS", bufs=2, space="PSUM") as psS,
            tc.tile_pool(name="psT", bufs=2, space="PSUM") as psT,
            tc.tile_pool(name="psO", bufs=2, space="PSUM") as psO,
        ):
            # ---- constants (loaded once, used by all iters) ----
            ident = const.tile([128, 128], F32, tag="ident", name="ident")
            make_identity(nc, ident[:])

            wqT = [const.tile([128, 2 * DIM], F32R, tag=f"wqT{c}", name=f"wqT{c}") for c in range(2)]
            wkT = [const.tile([128, 2 * DIM], F32R, tag=f"wkT{c}", name=f"wkT{c}") for c in range(2)]
            wvT = [const.tile([128, DIM], F32R, tag=f"wvT{c}", name=f"wvT{c}") for c in range(2)]
            woT = [const.tile([128, DIM], F32R, tag=f"woT{c}", name=f"woT{c}") for c in range(2)]
            fc1T = [const.tile([128, 4 * DIM], F32R, tag=f"fc1T{c}", name=f"fc1T{c}") for c in range(2)]
            fc2T = [const.tile([128, DIM], F32R, tag=f"fc2T{c}", name=f"fc2T{c}") for c in range(8)]
            for c in range(2):
                nc.sync.dma_start(wqT[c][:], d_wqT[ts(c, 128), :])
                nc.sync.dma_start(wkT[c][:], d_wkT[ts(c, 128), :])
                nc.sync.dma_start(wvT[c][:], d_wvT[ts(c, 128), :])
                nc.sync.dma_start(woT[c][:], d_woT[ts(c, 128), :])
                nc.sync.dma_start(fc1T[c][:], d_fc1T[ts(c, 128), :])
            for c in range(8):
                nc.sync.dma_start(fc2T[c][:], d_fc2T[ts(c, 128), :])

            gamma_bc = const.tile([128, DIM], F32, tag="gamma_bc", name="gamma_bc")
            beta_bc = const.tile([128, DIM], F32, tag="beta_bc", name="beta_bc")
            fc1b_bc = const.tile([128, 4 * DIM], F32, tag="fc1b_bc", name="fc1b_bc")
            fc2b_bc = const.tile([128, DIM], F32, tag="fc2b_bc", name="fc2b_bc")
            lam_rep = const.tile([128, DIM], F32, tag="lam_rep", name="lam_rep")
            nc.gpsimd.dma_start(out=gamma_bc[:], in_=bcast_ap(d_gamma))
            nc.gpsimd.dma_start(out=beta_bc[:], in_=bcast_ap(d_beta))
            nc.gpsimd.dma_start(out=fc1b_bc[:], in_=bcast_ap(d_fc1b))
            nc.gpsimd.dma_start(out=fc2b_bc[:], in_=bcast_ap(d_fc2b))
            nc.sync.dma_start(lam_rep[:], d_lam[:, :])

            # ---- persistent per-iter buffers (shared across iters) ----
            drift_s = [const.tile([128, DIM], F32, tag=f"drift{q}", name=f"drift{q}") for q in range(QT_N)]
            xT = [const.tile([128, QR], F32R, tag=f"xT{c}", name=f"xT{c}") for c in range(2)]
            oceanT = [const.tile([128, No], F32R, tag=f"oceanT{c}", name=f"oceanT{c}") for c in range(2)]
            QTm = [const.tile([MT_W[m], QR], F32R, tag=f"QT{m}", name=f"QT{m}") for m in range(6)]
            KTm = [const.tile([MT_W[m], No], F32R, tag=f"KT{m}", name=f"KT{m}") for m in range(6)]
            vp_dt = F32R if ATT_V2 else UV_DT
            Vp = [const.tile([128, H, 66], vp_dt, tag=f"Vp{k}", name=f"Vp{k}") for k in range(KT_N)]
            out_t = None
            ones_bc = None
            ones_col = const.tile([128, H], F32, tag="ones_col", name="ones_col")
            nc.vector.memset(ones_col[:], 1.0)
            if ATT_V2:
                ones_bc = const.tile([1, 32], F32, tag="ones_bc", name="ones_bc")
                nc.vector.memset(ones_bc[:], 1.0)
            else:
                out_t = [const.tile([128, DIM], F32, tag=f"attout{q}", name=f"attout{q}") for q in range(QT_N)]
            drift1 = [const.tile([128, DIM], F32, tag=f"drift1_{q}", name=f"drift1_{q}") for q in range(QT_N)]
            # SBUF reuse: xT is dead after QT build -> reuse for outT and yT;
            # oceanT is dead after KT/V build -> reuse as the 8 hT chunks.
            outT = xT
            yT = xT
            hT = [oceanT[c // 4][:, ts(c % 4, 512)] for c in range(8)]

            skip_scores = "scores" in skip
            skip_exp = "exp" in skip
            skip_uv = "uv" in skip
            ps_const = None
            ut_const = None
            if skip_scores:
                ps_const = psS.tile([128, 2, 512], F32, tag="psS", name="psS")
                nc.vector.memset(ps_const[:], 0.01)
            if skip_exp:
                u_dt = F32R if ATT_V2 else UV_DT
                ut_const = upool.tile([128, 2, 512], u_dt, tag="U", name="U")
                nc.vector.memset(ut_const[:], 1.0)
            if skip_uv and out_t is not None:
                for q in range(QT_N):
                    nc.vector.memset(out_t[q][:], 0.01)

            def pe_transpose(dst, src_tile, col, qidx):
                pt = psT.tile([128, 128], F32, tag="psT", name="psT")
                nc.tensor.transpose(pt[:], src_tile[:, ts(col, 128)], ident[:])
                nc.vector.tensor_copy(out=dst[:, ts(qidx, 128)], in_=pt[:])

            for _ in range(iters):
                # ---- load drift slice ----
                for q in range(QT_N):
                    nc.sync.dma_start(drift_s[q][:], d_drift[ts(q, 128), :])

                # ---- LN1 -> x, transpose -> xT ----
                x = [work.tile([128, DIM], F32, tag="x", name="x") for _ in range(QT_N)]
                _layernorm(nc, small, x, drift_s, gamma_bc, beta_bc)
                for q in range(QT_N):
                    for c in range(2):
                        pe_transpose(xT[c], x[q], c, q)

                # ---- load ocean + transpose -> oceanT ----
                for k in range(KT_N):
                    oc = work.tile([128, DIM], F32, tag="ocean", name="ocean")
                    nc.sync.dma_start(oc[:], d_ocean[ts(k, 128), :])
                    for c in range(2):
                        pe_transpose(oceanT[c], oc, c, k)

                # ---- QT = Wq @ xT  (6 tiles of (96|32, 512)) ----
                for m in range(6):
                    w = MT_W[m]
                    pp = psS.tile([128, 2, 512], F32, tag="psS", name="psS")
                    for c in range(2):
                        nc.tensor.matmul(
                            pp[:w, 0, :],
                            (wqT[c][:, ds(MT_O[m], w)]),
                            (xT[c][:]),
                            start=(c == 0),
                            stop=(c == 1),
                        )
                    nc.vector.tensor_copy(out=QTm[m][:], in_=pp[:w, 0, :])

                # ---- KT = Wk @ oceanT (6 tiles of (96|32, 2048)) ----
                for m in range(6):
                    w = MT_W[m]
                    for j in range(4):
                        pp = psS.tile([128, 2, 512], F32, tag="psS", name="psS")
                        for c in range(2):
                            nc.tensor.matmul(
                                pp[:w, 0, :],
                                (wkT[c][:, ds(MT_O[m], w)]),
                                (oceanT[c][:, ts(j, 512)]),
                                start=(c == 0),
                                stop=(c == 1),
                            )
                        nc.vector.tensor_copy(
                            out=KTm[m][:, ts(j, 512)], in_=pp[:w, 0, :]
                        )

                # ---- V' tiles: (128, 8 heads, 66) per k-tile ----
                # cols [0:32]=V_h, [32]=1, [33:65]=lam_h*V_h, [65]=1
                for k in range(KT_N):
                    pp = psS.tile([128, 2, 512], F32, tag="psS", name="psS")
                    pv = pp[:, 0, :DIM]
                    for c in range(2):
                        nc.tensor.matmul(
                            pv,
                            (oceanT[c][:, ts(k, 128)]),
                            (wvT[c][:]),
                            start=(c == 0),
                            stop=(c == 1),
                        )
                    pv3 = pp[:, 0, :DIM].rearrange("p (h d) -> p h d", h=H)
                    nc.vector.tensor_copy(out=Vp[k][:, :, 0:32], in_=pv3)
                    nc.vector.tensor_tensor(
                        out=Vp[k][:, :, 33:65],
                        in0=pv3,
                        in1=lam_rep[:].rearrange("p (h d) -> p h d", h=H),
                        op=OP.mult,
                    )
                    nc.vector.tensor_copy(
                        out=Vp[k][:, :, 32:33], in_=ones_col[:, :, None]
                    )
                    nc.vector.tensor_copy(
                        out=Vp[k][:, :, 65:66], in_=ones_col[:, :, None]
                    )

                # ---- attention per head (V2: V' stationary, out^T direct) ----
                if ATT_V2:
                    for h in range(H):
                        po = None
                        if not skip_uv:
                            po = [
                                psO.tile([33, 512], F32, tag="psO", name="psO")
                                for _ in range(2)
                            ]
                        for A in range(2):
                            n = 2 * h + A
                            m, roff = n // 3, 32 * (n % 3)
                            for kb in range(KT_N // 2):
                                ps = ps_const if skip_scores else psS.tile(
                                    [128, 2, 512], F32, tag="psS", name="psS"
                                )
                                if not skip_scores:
                                    for j in range(2):
                                        k = 2 * kb + j
                                        nc.tensor.matmul(
                                            ps[:, j, :],
                                            KTm[m][ds(roff, 32), ts(k, 128)],
                                            QTm[m][ds(roff, 32), :],
                                            start=True,
                                            stop=True,
                                        )
                                ut = ut_const if skip_exp else upool.tile(
                                    [128, 2, 512], F32R, tag="U", name="U"
                                )
                                if not skip_exp:
                                    nc.scalar.activation(
                                        out=ut[:], in_=ps[:], func=AF.Exp, scale=SCALE
                                    )
                                if not skip_uv:
                                    for j in range(2):
                                        k = 2 * kb + j
                                        nc.tensor.matmul(
                                            po[A][:, :],
                                            Vp[k][:, h, ds(33 * A, 33)],
                                            ut[:, j, :],
                                            start=(k == 0),
                                            stop=(k == KT_N - 1),
                                        )
                        if skip_uv:
                            continue
                        # combine into outT rows [32h:32h+32]:
                        # out^T = num1*r1 - num2*r2 (num2 carries lam already)
                        rr = med.tile([1, 2, 512], F32, tag="rr", name="rr")
                        nc.vector.reciprocal(out=rr[:, 0, :], in_=po[0][32:33, :])
                        nc.vector.reciprocal(out=rr[:, 1, :], in_=po[1][32:33, :])
                        pb = psS.tile([128, 2, 512], F32, tag="psS", name="psS")
                        nc.tensor.matmul(
                            pb[0:32, 0, :], ones_bc[:], rr[:, 0, :],
                            start=True, stop=True,
                        )
                        nc.tensor.matmul(
                            pb[0:32, 1, :], ones_bc[:], rr[:, 1, :],
                            start=True, stop=True,
                        )
                        rb = med.tile([32, 2, 512], F32, tag="rb", name="rb")
                        nc.vector.tensor_copy(out=rb[:], in_=pb[0:32, :, :])
                        tmp2 = med.tile([32, 512], F32, tag="t2", name="t2")
                        nc.vector.tensor_tensor(
                            out=tmp2[:], in0=po[1][0:32, :], in1=rb[:, 1, :],
                            op=OP.mult,
                        )
                        outh = med.tile([32, 512], F32R, tag="outh", name="outh")
                        nc.vector.tensor_tensor(
                            out=outh[:], in0=po[0][0:32, :], in1=rb[:, 0, :],
                            op=OP.mult,
                        )
                        nc.vector.tensor_tensor(
                            out=outh[:], in0=outh[:], in1=tmp2[:], op=OP.subtract
                        )
                        # partition-shift into outT rows [32h % 128] via DMA
                        # (DVE ops cannot write across a differing base partition)
                        nc.sync.dma_start(
                            outT[h // 4][ds(32 * (h % 4), 32), :], outh[:]
                        )
                for h in range(H):
                    if ATT_V2:
                        break
                    po = None
                    if not skip_uv:
                        po = psO.tile([128, 4 * 66], F32, tag="psO", name="psO")
                    for A in range(2):
                        n = 2 * h + A
                        m, roff = n // 3, 32 * (n % 3)
                        voff = 33 * A
                        for kb in range(KT_N // 2):  # batches of 2 k-tiles
                            ps = ps_const if skip_scores else psS.tile(
                                [128, 2, 512], F32, tag="psS", name="psS"
                            )
                            if not skip_scores:
                                for j in range(2):
                                    k = 2 * kb + j
                                    nc.tensor.matmul(
                                        ps[:, j, :],
                                        (KTm[m][ds(roff, 32), ts(k, 128)]),
                                        (QTm[m][ds(roff, 32), :]),
                                        start=True,
                                        stop=True,
                                    )
                            ut = ut_const if skip_exp else upool.tile(
                                [128, 2, 512], UV_DT, tag="U", name="U"
                            )
                            if not skip_exp:
                                nc.scalar.activation(
                                    out=ut[:], in_=ps[:], func=AF.Exp, scale=SCALE
                                )
                            for j in range(2):
                                if skip_uv:
                                    break
                                k = 2 * kb + j
                                for q in range(QT_N):
                                    # one accumulation group for the whole bank:
                                    # start zeroes the full 2KB zero-region, so
                                    # only the very first matmul starts and only
                                    # the very last stops.
                                    nc.tensor.matmul(
                                        po[:, ds(66 * q + voff, 33)],
                                        ut[:, j, ts(q, 128)],
                                        Vp[k][:, h, ds(voff, 33)],
                                        start=(A == 0 and k == 0 and q == 0),
                                        stop=(
                                            A == 1
                                            and k == KT_N - 1
                                            and q == QT_N - 1
                                        ),
                                    )
                    # combine: out_h = U1V*r1 - lamU2V*r2
                    for q in range(QT_N):
                        if skip_uv:
                            break
                        r1 = small.tile([128, 1], F32, tag="r1", name="r1")
                        r2 = small.tile([128, 1], F32, tag="r2", name="r2")
                        nc.vector.reciprocal(out=r1[:], in_=po[:, ds(66 * q + 32, 1)])
                        nc.vector.reciprocal(out=r2[:], in_=po[:, ds(66 * q + 65, 1)])
                        t2 = small.tile([128, 32], F32, tag="t2", name="t2")
                        nc.vector.tensor_scalar_mul(
                            out=t2[:], in0=po[:, ds(66 * q + 33, 32)], scalar1=r2[:]
                        )
                        dst = out_t[q][:, ts(h, 32)]
                        nc.vector.tensor_scalar_mul(
                            out=dst, in0=po[:, ds(66 * q, 32)], scalar1=r1[:]
                        )
                        nc.vector.tensor_tensor(
                            out=dst, in0=dst, in1=t2[:], op=OP.subtract
                        )

                # ---- transpose out -> outT (V1 only; V2 wrote outT directly) ----
                if not ATT_V2:
                    for q in range(QT_N):
                        for c in range(2):
                            pe_transpose(outT[c], out_t[q], c, q)

                # ---- drift1 = drift + out @ WoT ----
                for q in range(QT_N):
                    pp = psS.tile([128, 2, 512], F32, tag="psS", name="psS")
                    pw = pp[:, 0, :DIM]
                    for c in range(2):
                        nc.tensor.matmul(
                            pw,
                            (outT[c][:, ts(q, 128)]),
                            (woT[c][:]),
                            start=(c == 0),
                            stop=(c == 1),
                        )
                    nc.vector.tensor_add(out=drift1[q][:], in0=pw, in1=drift_s[q][:])

                # ---- LN2 -> y, transpose -> yT ----
                y = [work.tile([128, DIM], F32, tag="y", name="y") for _ in range(QT_N)]
                _layernorm(nc, small, y, drift1, gamma_bc, beta_bc)
                for q in range(QT_N):
                    for c in range(2):
                        pe_transpose(yT[c], y[q], c, q)

                # ---- fc1 + bias + gelu -> htmp, transpose -> hT ----
                for q in range(QT_N):
                    for n in range(2):
                        pp = psS.tile([128, 2, 512], F32, tag="psS", name="psS")
                        for c in range(2):
                            nc.tensor.matmul(
                                pp[:, 0, :],
                                (yT[c][:, ts(q, 128)]),
                                (fc1T[c][:, ts(n, 512)]),
                                start=(c == 0),
                                stop=(c == 1),
                            )
                        htmp = work.tile([128, 512], F32, tag="htmp", name="htmp")
                        nc.vector.tensor_add(
                            out=htmp[:],
                            in0=pp[:, 0, :],
                            in1=fc1b_bc[:, ts(n, 512)],
                        )
                        nc.scalar.activation(out=htmp[:], in_=htmp[:], func=AF.Gelu)
                        for cc in range(4):
                            pe_transpose(hT[4 * n + cc], htmp, cc, q)

                # ---- fc2 + bias + residual -> dout ----
                for q in range(QT_N):
                    pp = psS.tile([128, 2, 512], F32, tag="psS", name="psS")
                    pf = pp[:, 0, :DIM]
                    for c in range(8):
                        nc.tensor.matmul(
                            pf,
                            (hT[c][:, ts(q, 128)]),
                            (fc2T[c][:]),
                            start=(c == 0),
                            stop=(c == 7),
                        )
                    do = work.tile([128, DIM], F32, tag="dout_t", name="dout_t")
                    nc.vector.tensor_add(out=do[:], in0=pf, in1=drift1[q][:])
                    nc.vector.tensor_add(out=do[:], in0=do[:], in1=fc2b_bc[:])
                    nc.sync.dma_start(d_out[ts(q, 128), :], do[:])


def make_in_maps(inputs):
    """Host-side prep: slice/shard inputs per core, pre-transpose weights,
    compute lam, build broadcast helpers."""
    f = np.float32
    drift = np.ascontiguousarray(np.asarray(inputs["drift"], f))
    ocean = np.ascontiguousarray(np.asarray(inputs["ocean"], f))
    lq1, lk1 = np.asarray(inputs["lq1"], f), np.asarray(inputs["lk1"], f)
    lq2, lk2 = np.asarray(inputs["lq2"], f), np.asarray(inputs["lk2"], f)
    lam = (
        np.exp(np.sum(lq1 * lk1, -1)) - np.exp(np.sum(lq2 * lk2, -1)) + LAMBDA_INIT
    ).astype(f)  # (H,)
    lam_rep = np.tile(np.repeat(lam, HD)[None, :], (128, 1)).astype(f)  # (128, 256)

    shared = {
        "WqT": np.ascontiguousarray(np.asarray(inputs["Wq"], f).T),
        "WkT": np.ascontiguousarray(np.asarray(inputs["Wk"], f).T),
        "WvT": np.ascontiguousarray(np.asarray(inputs["Wv"], f).T),
        "WoT": np.ascontiguousarray(np.asarray(inputs["Wo"], f).T),
        "fc1T": np.ascontiguousarray(np.asarray(inputs["fc1_w"], f).T),
        "fc2T": np.ascontiguousarray(np.asarray(inputs["fc2_w"], f).T),
        "gamma": np.asarray(inputs["gamma"], f),
        "beta": np.asarray(inputs["beta"], f),
        "fc1_b": np.asarray(inputs["fc1_b"], f),
        "fc2_b": np.asarray(inputs["fc2_b"], f),
        "lam_rep": lam_rep,
    }
    in_maps = []
    for c in range(NCORES):
        b = c // (NCORES // B)
        q0 = (c % (NCORES // B)) * QR
        m = dict(shared)
        m["drift_s"] = np.ascontiguousarray(drift[b, q0 : q0 + QR])
        m["ocean_b"] = np.ascontiguousarray(ocean[b])
        in_maps.append(m)
    return in_maps


_CACHED_NC = None


def _get_nc():
    global _CACHED_NC
    if _CACHED_NC is None:
        nc = bacc.Bacc(
            "TRN2",
            target_bir_lowering=False,
            debug=False,
            enable_asserts=False,
            num_devices=NCORES,
        )
        build_program(nc, iters=1)
        nc.compile()
        _CACHED_NC = nc
    return _CACHED_NC


def kernel(**inputs):
    nc = _get_nc()
    in_maps = make_in_maps(inputs)
    res = run_bass_kernel_spmd(nc, in_maps, core_ids=list(range(NCORES)))
    parts = [res.results[c]["dout"] for c in range(NCORES)]
    drift_out = np.stack(parts, 0).reshape(B, Nd, DIM)
    ocean_out = np.asarray(inputs["ocean"], np.float32)
    return drift_out, ocean_out


# revision 20
# speedup vs baseline: 1.4308x; 1.4308x over previous
"""Trainium2 Bass kernel for nn_Block_68822555951557 (differential cross-attention
transformer block + MLP).

Sharding: 8 cores; core c handles batch b=c//4 and query rows [(c%4)*512, +512).
Each core is fully independent (no collectives): it recomputes K/V for its batch
from ocean, runs all 8 heads for its 512 query rows, then Wo/LN/MLP on its rows.

Compute layout notes:
- Weights are pre-transposed on host and passed as extra inputs.
- Activation transposes (x, ocean, out, y, h) are done on-device with the PE.
- Attention scores are computed in S^T layout: psum[k=128, q=512] via
  lhsT = K^T d-slice (32,128), rhs = Q^T d-slice (32,512), float32r (1 cyc/row).
- exp on ScalarE with fused *SCALE, bf16 output; softmax denominators come for
  free from an appended ones-column in V' during the U@V' matmul.
- att@V: out[q,33] accumulated over 16 k-tiles into one packed PSUM bank per
  head: cols [66q+0:33]=U1@[V|1], [66q+33:66]=U2@[lam*V|1] for each q-tile.
- Combine per (head, qtile): out = U1V*r1 - lamU2V*r2, r=1/s from ones cols.
"""

import sys

if "/opt/trn_rl_repo" not in sys.path:
    sys.path.insert(0, "/opt/trn_rl_repo")

import numpy as np

import concourse.bass as bass
import concourse.mybir as mybir
import concourse.tile as tile
from concourse import bacc
from concourse.bass import ds, ts
from concourse.bass_utils import run_bass_kernel_spmd
from concourse.masks import make_identity

F32 = mybir.dt.float32
F32R = mybir.dt.float32r
BF16 = mybir.dt.bfloat16
AF = mybir.ActivationFunctionType
OP = mybir.AluOpType

DIM = 256
H = 8
HD = 32
EPS = 1e-5
LAMBDA_INIT = 0.1
SCALE = HD ** -0.5
B, Nd, No = 2, 2048, 2048
NCORES = 8
QR = (B * Nd) // NCORES  # 512 query rows per core
QT_N = QR // 128  # 4 q tiles
KT_N = No // 128  # 16 k tiles

# dtype for the U (=exp(S)) and V' operands of the att@V matmul.
UV_DT = F32
# ATT_V2: att@V with V' stationary producing out^T directly (f32r, 1cyc/row,
# 4x fewer matmuls and no per-matmul 128x128 weight reload).
ATT_V2 = True




def _layernorm(nc, small, out_tiles, in_tiles, gamma_bc, beta_bc):
    """LayerNorm along free axis (256) for a list of (128,256) tiles."""
    for o, t in zip(out_tiles, in_tiles):
        stats = small.tile([128, 6], F32, tag="ln_stats", name="ln_stats")
        mv = small.tile([128, 2], F32, tag="ln_mv", name="ln_mv")
        nc.vector.bn_stats(out=stats[:], in_=t[:])
        nc.vector.bn_aggr(out=mv[:], in_=stats[:])
        eps_t = small.tile([128, 1], F32, tag="ln_eps", name="ln_eps")
        nc.vector.memset(eps_t[:], EPS)
        rstd = small.tile([128, 1], F32, tag="ln_rstd", name="ln_rstd")
        nc.scalar.activation(
            out=rstd[:], in_=mv[:, 1:2], func=AF.Sqrt, bias=eps_t[:], scale=1.0
        )
        nc.vector.reciprocal(out=rstd[:], in_=rstd[:])
        nc.vector.tensor_scalar(
            out=o[:],
            in0=t[:],
            scalar1=mv[:, 0:1],
            scalar2=rstd[:],
            op0=OP.subtract,
            op1=OP.mult,
        )
        nc.vector.tensor_mul(out=o[:], in0=o[:], in1=gamma_bc[:])
        nc.vector.tensor_add(out=o[:], in0=o[:], in1=beta_bc[:])


def build_program(nc, iters=1, skip=()):
    """Build the SPMD per-core program (tensors declared on nc).
    iters>1 repeats the body, reusing all buffers (for timing via slope)."""
    # ---- DRAM I/O ----
    d_drift = nc.dram_tensor("drift_s", (QR, DIM), F32, kind="ExternalInput")
    d_oceanT = nc.dram_tensor("oceanT_b", (DIM, No), F32R, kind="ExternalInput")
    d_wqT = nc.dram_tensor("WqT", (DIM, 2 * DIM), F32R, kind="ExternalInput")
    d_wkT = nc.dram_tensor("WkT", (DIM, 2 * DIM), F32R, kind="ExternalInput")
    d_wvT = nc.dram_tensor("WvT", (DIM, DIM), F32R, kind="ExternalInput")
    d_woT = nc.dram_tensor("WoT", (DIM, DIM), F32R, kind="ExternalInput")
    d_fc1T = nc.dram_tensor("fc1T", (DIM, 4 * DIM), F32R, kind="ExternalInput")
    d_fc2T = nc.dram_tensor("fc2T", (4 * DIM, DIM), F32R, kind="ExternalInput")
    d_gamma = nc.dram_tensor("gamma", (DIM,), F32, kind="ExternalInput")
    d_beta = nc.dram_tensor("beta", (DIM,), F32, kind="ExternalInput")
    d_fc1b = nc.dram_tensor("fc1_b", (4 * DIM,), F32, kind="ExternalInput")
    d_fc2b = nc.dram_tensor("fc2_b", (DIM,), F32, kind="ExternalInput")
    d_lam = nc.dram_tensor("lam_rep", (128, DIM), F32, kind="ExternalInput")
    d_out = nc.dram_tensor("dout", (QR, DIM), F32, kind="ExternalOutput")

    def bcast_ap(d1, p=128):
        """DRAM 1-D tensor -> (p, n) AP with partition step 0 (row broadcast)."""
        a = d1.ap()
        return bass.AP(tensor=a.tensor, offset=a.offset, ap=[[0, p], a.ap[0]])

    # Q/K head-dim tiling: Qdim (512) split into 96-row projection tiles so
    # every 32-row (head, half) slice starts at partition 0/32/64 (HW limit).
    # block n = 2h+A lives in tile n//3 at offset 32*(n%3).
    MT_W = [96, 96, 96, 96, 96, 32]
    MT_O = [0, 96, 192, 288, 384, 480]

    with tile.TileContext(nc) as tc:
        with (
            tc.tile_pool(name="const", bufs=1) as const,
            tc.tile_pool(name="work", bufs=2) as work,
            tc.tile_pool(name="small", bufs=4) as small,
            tc.tile_pool(name="med", bufs=1) as med,
            tc.tile_pool(name="upool", bufs=2) as upool,
            tc.tile_pool(name="psS", bufs=2, space="PSUM") as psS,
            tc.tile_pool(name="psT", bufs=2, space="PSUM") as psT,
            tc.tile_pool(name="psO", bufs=2, space="PSUM") as psO,
        ):
            # ---- constants (loaded once, used by all iters) ----
            ident = const.tile([128, 128], F32, tag="ident", name="ident")
            make_identity(nc, ident[:])

            wqT = [const.tile([128, 2 * DIM], F32R, tag=f"wqT{c}", name=f"wqT{c}") for c in range(2)]
            wkT = [const.tile([128, 2 * DIM], F32R, tag=f"wkT{c}", name=f"wkT{c}") for c in range(2)]
            wvT = [const.tile([128, DIM], F32R, tag=f"wvT{c}", name=f"wvT{c}") for c in range(2)]
            woT = [const.tile([128, DIM], F32R, tag=f"woT{c}", name=f"woT{c}") for c in range(2)]
            fc1T = [const.tile([128, 4 * DIM], F32R, tag=f"fc1T{c}", name=f"fc1T{c}") for c in range(2)]
            fc2T = [const.tile([128, DIM], F32R, tag=f"fc2T{c}", name=f"fc2T{c}") for c in range(8)]
            for c in range(2):
                nc.sync.dma_start(wqT[c][:], d_wqT[ts(c, 128), :])
                nc.sync.dma_start(wkT[c][:], d_wkT[ts(c, 128), :])
                nc.sync.dma_start(wvT[c][:], d_wvT[ts(c, 128), :])
                nc.sync.dma_start(woT[c][:], d_woT[ts(c, 128), :])
                nc.sync.dma_start(fc1T[c][:], d_fc1T[ts(c, 128), :])
            for c in range(8):
                nc.sync.dma_start(fc2T[c][:], d_fc2T[ts(c, 128), :])

            gamma_bc = const.tile([128, DIM], F32, tag="gamma_bc", name="gamma_bc")
            beta_bc = const.tile([128, DIM], F32, tag="beta_bc", name="beta_bc")
            fc1b_col = const.tile([128, 8], F32, tag="fc1b_col", name="fc1b_col")
            fc2b_bc = const.tile([128, DIM], F32, tag="fc2b_bc", name="fc2b_bc")
            lam_rep = const.tile([128, DIM], F32, tag="lam_rep", name="lam_rep")
            nc.gpsimd.dma_start(out=gamma_bc[:], in_=bcast_ap(d_gamma))
            nc.gpsimd.dma_start(out=beta_bc[:], in_=bcast_ap(d_beta))
            nc.sync.dma_start(
                fc1b_col[:], d_fc1b.ap().rearrange("(c p) -> p c", p=128)
            )
            nc.gpsimd.dma_start(out=fc2b_bc[:], in_=bcast_ap(d_fc2b))
            nc.sync.dma_start(lam_rep[:], d_lam[:, :])

            # ---- persistent per-iter buffers (shared across iters) ----
            drift_s = [const.tile([128, DIM], F32, tag=f"drift{q}", name=f"drift{q}") for q in range(QT_N)]
            xT = [const.tile([128, QR], F32R, tag=f"xT{c}", name=f"xT{c}") for c in range(2)]
            oceanT = [const.tile([128, No], F32R, tag=f"oceanT{c}", name=f"oceanT{c}") for c in range(2)]
            QTm = [const.tile([MT_W[m], QR], F32R, tag=f"QT{m}", name=f"QT{m}") for m in range(6)]
            KTm = [const.tile([MT_W[m], No], F32R, tag=f"KT{m}", name=f"KT{m}") for m in range(6)]
            vp_dt = F32R if ATT_V2 else UV_DT
            Vp = [const.tile([128, H, 66], vp_dt, tag=f"Vp{k}", name=f"Vp{k}") for k in range(KT_N)]
            out_t = None
            ones_bc = None
            ones_col = const.tile([128, H], F32, tag="ones_col", name="ones_col")
            nc.vector.memset(ones_col[:], 1.0)
            if ATT_V2:
                ones_bc = const.tile([1, 32], F32, tag="ones_bc", name="ones_bc")
                nc.vector.memset(ones_bc[:], 1.0)
            else:
                out_t = [const.tile([128, DIM], F32, tag=f"attout{q}", name=f"attout{q}") for q in range(QT_N)]
            drift1 = [const.tile([128, DIM], F32, tag=f"drift1_{q}", name=f"drift1_{q}") for q in range(QT_N)]
            # SBUF reuse: xT is dead after QT build -> reuse for outT and yT;
            # oceanT is dead after KT/V build -> reuse as the 8 hT chunks.
            outT = xT
            yT = xT
            hT = [oceanT[c // 4][:, ts(c % 4, 512)] for c in range(8)]

            skip_scores = "scores" in skip
            skip_exp = "exp" in skip
            skip_uv = "uv" in skip
            ps_const = None
            ut_const = None
            if skip_scores:
                ps_const = psS.tile([128, 2, 512], F32, tag="psS", name="psS")
                nc.vector.memset(ps_const[:], 0.01)
            if skip_exp:
                u_dt = F32R if ATT_V2 else UV_DT
                ut_const = upool.tile([128, 2, 512], u_dt, tag="U", name="U")
                nc.vector.memset(ut_const[:], 1.0)
            if skip_uv and out_t is not None:
                for q in range(QT_N):
                    nc.vector.memset(out_t[q][:], 0.01)

            def pe_transpose(dst, src_tile, col, qidx):
                pt = psT.tile([128, 128], F32, tag="psT", name="psT")
                nc.tensor.transpose(pt[:], src_tile[:, ts(col, 128)], ident[:])
                nc.vector.tensor_copy(out=dst[:, ts(qidx, 128)], in_=pt[:])

            for _ in range(iters):
                # ---- load drift slice ----
                for q in range(QT_N):
                    nc.sync.dma_start(drift_s[q][:], d_drift[ts(q, 128), :])

                # ---- LN1 -> x, transpose -> xT ----
                x = [work.tile([128, DIM], F32, tag="x", name="x") for _ in range(QT_N)]
                _layernorm(nc, small, x, drift_s, gamma_bc, beta_bc)
                for q in range(QT_N):
                    for c in range(2):
                        pe_transpose(xT[c], x[q], c, q)

                # ---- load oceanT directly (host-pretransposed) ----
                for c in range(2):
                    nc.sync.dma_start(oceanT[c][:], d_oceanT[ts(c, 128), :])

                # ---- QT = Wq @ xT  (6 tiles of (96|32, 512)) ----
                for m in range(6):
                    w = MT_W[m]
                    pp = psS.tile([128, 2, 512], F32, tag="psS", name="psS")
                    for c in range(2):
                        nc.tensor.matmul(
                            pp[:w, 0, :],
                            (wqT[c][:, ds(MT_O[m], w)]),
                            (xT[c][:]),
                            start=(c == 0),
                            stop=(c == 1),
                        )
                    nc.vector.tensor_copy(out=QTm[m][:], in_=pp[:w, 0, :])

                # ---- KT = Wk @ oceanT (6 tiles of (96|32, 2048)) ----
                for m in range(6):
                    w = MT_W[m]
                    for j in range(4):
                        pp = psS.tile([128, 2, 512], F32, tag="psS", name="psS")
                        for c in range(2):
                            nc.tensor.matmul(
                                pp[:w, 0, :],
                                (wkT[c][:, ds(MT_O[m], w)]),
                                (oceanT[c][:, ts(j, 512)]),
                                start=(c == 0),
                                stop=(c == 1),
                            )
                        nc.vector.tensor_copy(
                            out=KTm[m][:, ts(j, 512)], in_=pp[:w, 0, :]
                        )

                # ---- V' tiles: (128, 8 heads, 66) per k-tile ----
                # cols [0:32]=V_h, [32]=1, [33:65]=lam_h*V_h, [65]=1
                for k in range(KT_N):
                    pp = psS.tile([128, 2, 512], F32, tag="psS", name="psS")
                    pv = pp[:, 0, :DIM]
                    for c in range(2):
                        nc.tensor.matmul(
                            pv,
                            (oceanT[c][:, ts(k, 128)]),
                            (wvT[c][:]),
                            start=(c == 0),
                            stop=(c == 1),
                        )
                    pv3 = pp[:, 0, :DIM].rearrange("p (h d) -> p h d", h=H)
                    nc.vector.tensor_copy(out=Vp[k][:, :, 0:32], in_=pv3)
                    nc.vector.tensor_tensor(
                        out=Vp[k][:, :, 33:65],
                        in0=pv3,
                        in1=lam_rep[:].rearrange("p (h d) -> p h d", h=H),
                        op=OP.mult,
                    )
                    nc.vector.tensor_copy(
                        out=Vp[k][:, :, 32:33], in_=ones_col[:, :, None]
                    )
                    nc.vector.tensor_copy(
                        out=Vp[k][:, :, 65:66], in_=ones_col[:, :, None]
                    )

                # ---- attention per head (V2: V' stationary, out^T direct) ----
                if ATT_V2:
                    for h in range(H):
                        po = None
                        if not skip_uv:
                            po = [
                                psO.tile([33, 512], F32, tag="psO", name="psO")
                                for _ in range(2)
                            ]
                        for A in range(2):
                            n = 2 * h + A
                            m, roff = n // 3, 32 * (n % 3)
                            for kb in range(KT_N // 2):
                                ps = ps_const if skip_scores else psS.tile(
                                    [128, 2, 512], F32, tag="psS", name="psS"
                                )
                                if not skip_scores:
                                    for j in range(2):
                                        k = 2 * kb + j
                                        nc.tensor.matmul(
                                            ps[:, j, :],
                                            KTm[m][ds(roff, 32), ts(k, 128)],
                                            QTm[m][ds(roff, 32), :],
                                            start=True,
                                            stop=True,
                                        )
                                ut = ut_const if skip_exp else upool.tile(
                                    [128, 2, 512], F32R, tag="U", name="U"
                                )
                                if not skip_exp:
                                    nc.scalar.activation(
                                        out=ut[:], in_=ps[:], func=AF.Exp, scale=SCALE
                                    )
                                if not skip_uv:
                                    for j in range(2):
                                        k = 2 * kb + j
                                        nc.tensor.matmul(
                                            po[A][:, :],
                                            Vp[k][:, h, ds(33 * A, 33)],
                                            ut[:, j, :],
                                            start=(k == 0),
                                            stop=(k == KT_N - 1),
                                        )
                        if skip_uv:
                            continue
                        # combine into outT rows [32h:32h+32]:
                        # out^T = num1*r1 - num2*r2 (num2 carries lam already)
                        rr = med.tile([1, 2, 512], F32, tag="rr", name="rr")
                        nc.vector.reciprocal(out=rr[:, 0, :], in_=po[0][32:33, :])
                        nc.vector.reciprocal(out=rr[:, 1, :], in_=po[1][32:33, :])
                        pb = psS.tile([128, 2, 512], F32, tag="psS", name="psS")
                        nc.tensor.matmul(
                            pb[0:32, 0, :], ones_bc[:], rr[:, 0, :],
                            start=True, stop=True,
                        )
                        nc.tensor.matmul(
                            pb[0:32, 1, :], ones_bc[:], rr[:, 1, :],
                            start=True, stop=True,
                        )
                        rb = med.tile([32, 2, 512], F32, tag="rb", name="rb")
                        nc.vector.tensor_copy(out=rb[:], in_=pb[0:32, :, :])
                        tmp2 = med.tile([32, 512], F32, tag="t2", name="t2")
                        nc.vector.tensor_tensor(
                            out=tmp2[:], in0=po[1][0:32, :], in1=rb[:, 1, :],
                            op=OP.mult,
                        )
                        outh = med.tile([32, 512], F32R, tag="outh", name="outh")
                        nc.vector.tensor_tensor(
                            out=outh[:], in0=po[0][0:32, :], in1=rb[:, 0, :],
                            op=OP.mult,
                        )
                        nc.vector.tensor_tensor(
                            out=outh[:], in0=outh[:], in1=tmp2[:], op=OP.subtract
                        )
                        # partition-shift into outT rows [32h % 128] via DMA
                        # (DVE ops cannot write across a differing base partition)
                        nc.sync.dma_start(
                            outT[h // 4][ds(32 * (h % 4), 32), :], outh[:]
                        )
                for h in range(H):
                    if ATT_V2:
                        break
                    po = None
                    if not skip_uv:
                        po = psO.tile([128, 4 * 66], F32, tag="psO", name="psO")
                    for A in range(2):
                        n = 2 * h + A
                        m, roff = n // 3, 32 * (n % 3)
                        voff = 33 * A
                        for kb in range(KT_N // 2):  # batches of 2 k-tiles
                            ps = ps_const if skip_scores else psS.tile(
                                [128, 2, 512], F32, tag="psS", name="psS"
                            )
                            if not skip_scores:
                                for j in range(2):
                                    k = 2 * kb + j
                                    nc.tensor.matmul(
                                        ps[:, j, :],
                                        (KTm[m][ds(roff, 32), ts(k, 128)]),
                                        (QTm[m][ds(roff, 32), :]),
                                        start=True,
                                        stop=True,
                                    )
                            ut = ut_const if skip_exp else upool.tile(
                                [128, 2, 512], UV_DT, tag="U", name="U"
                            )
                            if not skip_exp:
                                nc.scalar.activation(
                                    out=ut[:], in_=ps[:], func=AF.Exp, scale=SCALE
                                )
                            for j in range(2):
                                if skip_uv:
                                    break
                                k = 2 * kb + j
                                for q in range(QT_N):
                                    # one accumulation group for the whole bank:
                                    # start zeroes the full 2KB zero-region, so
                                    # only the very first matmul starts and only
                                    # the very last stops.
                                    nc.tensor.matmul(
                                        po[:, ds(66 * q + voff, 33)],
                                        ut[:, j, ts(q, 128)],
                                        Vp[k][:, h, ds(voff, 33)],
                                        start=(A == 0 and k == 0 and q == 0),
                                        stop=(
                                            A == 1
                                            and k == KT_N - 1
                                            and q == QT_N - 1
                                        ),
                                    )
                    # combine: out_h = U1V*r1 - lamU2V*r2
                    for q in range(QT_N):
                        if skip_uv:
                            break
                        r1 = small.tile([128, 1], F32, tag="r1", name="r1")
                        r2 = small.tile([128, 1], F32, tag="r2", name="r2")
                        nc.vector.reciprocal(out=r1[:], in_=po[:, ds(66 * q + 32, 1)])
                        nc.vector.reciprocal(out=r2[:], in_=po[:, ds(66 * q + 65, 1)])
                        t2 = small.tile([128, 32], F32, tag="t2", name="t2")
                        nc.vector.tensor_scalar_mul(
                            out=t2[:], in0=po[:, ds(66 * q + 33, 32)], scalar1=r2[:]
                        )
                        dst = out_t[q][:, ts(h, 32)]
                        nc.vector.tensor_scalar_mul(
                            out=dst, in0=po[:, ds(66 * q, 32)], scalar1=r1[:]
                        )
                        nc.vector.tensor_tensor(
                            out=dst, in0=dst, in1=t2[:], op=OP.subtract
                        )

                # ---- transpose out -> outT (V1 only; V2 wrote outT directly) ----
                if not ATT_V2:
                    for q in range(QT_N):
                        for c in range(2):
                            pe_transpose(outT[c], out_t[q], c, q)

                # ---- drift1 = drift + out @ WoT ----
                for q in range(QT_N):
                    pp = psS.tile([128, 2, 512], F32, tag="psS", name="psS")
                    pw = pp[:, 0, :DIM]
                    for c in range(2):
                        nc.tensor.matmul(
                            pw,
                            (outT[c][:, ts(q, 128)]),
                            (woT[c][:]),
                            start=(c == 0),
                            stop=(c == 1),
                        )
                    nc.vector.tensor_add(out=drift1[q][:], in0=pw, in1=drift_s[q][:])

                # ---- LN2 -> y, transpose -> yT ----
                y = [work.tile([128, DIM], F32, tag="y", name="y") for _ in range(QT_N)]
                _layernorm(nc, small, y, drift1, gamma_bc, beta_bc)
                for q in range(QT_N):
                    for c in range(2):
                        pe_transpose(yT[c], y[q], c, q)

                # ---- fc1 -> h^T directly: h^T = fc1_w @ y^T, fused bias+gelu ----
                for t8 in range(8):
                    pp = psS.tile([128, 2, 512], F32, tag="psS", name="psS")
                    for c in range(2):
                        nc.tensor.matmul(
                            pp[:, 0, :],
                            fc1T[c][:, ts(t8, 128)],
                            yT[c][:],
                            start=(c == 0),
                            stop=(c == 1),
                        )
                    nc.scalar.activation(
                        out=hT[t8][:],
                        in_=pp[:, 0, :],
                        func=AF.Gelu,
                        bias=fc1b_col[:, t8 : t8 + 1],
                        scale=1.0,
                    )

                # ---- fc2 + bias + residual -> dout ----
                for q in range(QT_N):
                    pp = psS.tile([128, 2, 512], F32, tag="psS", name="psS")
                    pf = pp[:, 0, :DIM]
                    for c in range(8):
                        nc.tensor.matmul(
                            pf,
                            (hT[c][:, ts(q, 128)]),
                            (fc2T[c][:]),
                            start=(c == 0),
                            stop=(c == 7),
                        )
                    do = work.tile([128, DIM], F32, tag="dout_t", name="dout_t")
                    nc.vector.tensor_add(out=do[:], in0=pf, in1=drift1[q][:])
                    nc.vector.tensor_add(out=do[:], in0=do[:], in1=fc2b_bc[:])
                    nc.sync.dma_start(d_out[ts(q, 128), :], do[:])


def make_in_maps(inputs):
    """Host-side prep: slice/shard inputs per core, pre-transpose weights,
    compute lam, build broadcast helpers."""
    f = np.float32
    drift = np.ascontiguousarray(np.asarray(inputs["drift"], f))
    ocean = np.ascontiguousarray(np.asarray(inputs["ocean"], f))
    lq1, lk1 = np.asarray(inputs["lq1"], f), np.asarray(inputs["lk1"], f)
    lq2, lk2 = np.asarray(inputs["lq2"], f), np.asarray(inputs["lk2"], f)
    lam = (
        np.exp(np.sum(lq1 * lk1, -1)) - np.exp(np.sum(lq2 * lk2, -1)) + LAMBDA_INIT
    ).astype(f)  # (H,)
    lam_rep = np.tile(np.repeat(lam, HD)[None, :], (128, 1)).astype(f)  # (128, 256)

    shared = {
        "WqT": np.ascontiguousarray(np.asarray(inputs["Wq"], f).T),
        "WkT": np.ascontiguousarray(np.asarray(inputs["Wk"], f).T),
        "WvT": np.ascontiguousarray(np.asarray(inputs["Wv"], f).T),
        "WoT": np.ascontiguousarray(np.asarray(inputs["Wo"], f).T),
        "fc1T": np.ascontiguousarray(np.asarray(inputs["fc1_w"], f).T),
        "fc2T": np.ascontiguousarray(np.asarray(inputs["fc2_w"], f).T),
        "gamma": np.asarray(inputs["gamma"], f),
        "beta": np.asarray(inputs["beta"], f),
        "fc1_b": np.asarray(inputs["fc1_b"], f),
        "fc2_b": np.asarray(inputs["fc2_b"], f),
        "lam_rep": lam_rep,
    }
    oceanT = [np.ascontiguousarray(ocean[b].T) for b in range(B)]
    in_maps = []
    for c in range(NCORES):
        b = c // (NCORES // B)
        q0 = (c % (NCORES // B)) * QR
        m = dict(shared)
        m["drift_s"] = np.ascontiguousarray(drift[b, q0 : q0 + QR])
        m["oceanT_b"] = oceanT[b]
        in_maps.append(m)
    return in_maps


_CACHED_NC = None


def _get_nc():
    global _CACHED_NC
    if _CACHED_NC is None:
        nc = bacc.Bacc(
            "TRN2",
            target_bir_lowering=False,
            debug=False,
            enable_asserts=False,
            num_devices=NCORES,
        )
        build_program(nc, iters=1)
        nc.compile()
        _CACHED_NC = nc
    return _CACHED_NC


def kernel(**inputs):
    nc = _get_nc()
    in_maps = make_in_maps(inputs)
    res = run_bass_kernel_spmd(nc, in_maps, core_ids=list(range(NCORES)))
    parts = [res.results[c]["dout"] for c in range(NCORES)]
    drift_out = np.stack(parts, 0).reshape(B, Nd, DIM)
    ocean_out = np.asarray(inputs["ocean"], np.float32)
    return drift_out, ocean_out


# revision 21
# speedup vs baseline: 7.1043x; 4.9654x over previous
"""Trainium2 Bass kernel for nn_Block_68822555951557 (differential cross-attention
transformer block + MLP).

Sharding: 8 cores; core c handles batch b=c//4 and query rows [(c%4)*512, +512).
Each core is fully independent (no collectives): it recomputes K/V for its batch
from ocean, runs all 8 heads for its 512 query rows, then Wo/LN/MLP on its rows.

Compute layout notes:
- Weights are pre-transposed on host and passed as extra inputs.
- Activation transposes (x, ocean, out, y, h) are done on-device with the PE.
- Attention scores are computed in S^T layout: psum[k=128, q=512] via
  lhsT = K^T d-slice (32,128), rhs = Q^T d-slice (32,512), float32r (1 cyc/row).
- exp on ScalarE with fused *SCALE, bf16 output; softmax denominators come for
  free from an appended ones-column in V' during the U@V' matmul.
- att@V: out[q,33] accumulated over 16 k-tiles into one packed PSUM bank per
  head: cols [66q+0:33]=U1@[V|1], [66q+33:66]=U2@[lam*V|1] for each q-tile.
- Combine per (head, qtile): out = U1V*r1 - lamU2V*r2, r=1/s from ones cols.
"""

import sys

if "/opt/trn_rl_repo" not in sys.path:
    sys.path.insert(0, "/opt/trn_rl_repo")

import numpy as np

import concourse.bass as bass
import concourse.mybir as mybir
import concourse.tile as tile
from concourse import bacc
from concourse.bass import ds, ts
from concourse.bass_utils import run_bass_kernel_spmd
from concourse.masks import make_identity

F32 = mybir.dt.float32
F32R = mybir.dt.float32r
BF16 = mybir.dt.bfloat16
AF = mybir.ActivationFunctionType
OP = mybir.AluOpType

DIM = 256
H = 8
HD = 32
EPS = 1e-5
LAMBDA_INIT = 0.1
SCALE = HD ** -0.5
B, Nd, No = 2, 2048, 2048
NCORES = 8
QR = (B * Nd) // NCORES  # 512 query rows per core
QT_N = QR // 128  # 4 q tiles
KT_N = No // 128  # 16 k tiles

# dtype for the U (=exp(S)) and V' operands of the att@V matmul.
UV_DT = F32
# ATT_V2: att@V with V' stationary producing out^T directly (f32r, 1cyc/row,
# 4x fewer matmuls and no per-matmul 128x128 weight reload).
ATT_V2 = True




def _layernorm(nc, small, out_tiles, in_tiles, gamma_bc, beta_bc):
    """LayerNorm along free axis (256) for a list of (128,256) tiles."""
    for o, t in zip(out_tiles, in_tiles):
        stats = small.tile([128, 6], F32, tag="ln_stats", name="ln_stats")
        mv = small.tile([128, 2], F32, tag="ln_mv", name="ln_mv")
        nc.vector.bn_stats(out=stats[:], in_=t[:])
        nc.vector.bn_aggr(out=mv[:], in_=stats[:])
        eps_t = small.tile([128, 1], F32, tag="ln_eps", name="ln_eps")
        nc.vector.memset(eps_t[:], EPS)
        rstd = small.tile([128, 1], F32, tag="ln_rstd", name="ln_rstd")
        nc.scalar.activation(
            out=rstd[:], in_=mv[:, 1:2], func=AF.Sqrt, bias=eps_t[:], scale=1.0
        )
        nc.vector.reciprocal(out=rstd[:], in_=rstd[:])
        nc.vector.tensor_scalar(
            out=o[:],
            in0=t[:],
            scalar1=mv[:, 0:1],
            scalar2=rstd[:],
            op0=OP.subtract,
            op1=OP.mult,
        )
        nc.vector.tensor_mul(out=o[:], in0=o[:], in1=gamma_bc[:])
        nc.vector.tensor_add(out=o[:], in0=o[:], in1=beta_bc[:])


def build_program(nc, iters=1, skip=()):
    """Build the SPMD per-core program (tensors declared on nc).
    iters>1 repeats the body, reusing all buffers (for timing via slope)."""
    # ---- DRAM I/O ----
    d_drift = nc.dram_tensor("drift_s", (QR, DIM), F32, kind="ExternalInput")
    d_oceanT = nc.dram_tensor("oceanT_b", (DIM, No), F32R, kind="ExternalInput")
    d_wqT = nc.dram_tensor("WqT", (DIM, 2 * DIM), F32R, kind="ExternalInput")
    d_wkT = nc.dram_tensor("WkT", (DIM, 2 * DIM), F32R, kind="ExternalInput")
    d_wvT = nc.dram_tensor("WvT", (DIM, DIM), F32R, kind="ExternalInput")
    d_woT = nc.dram_tensor("WoT", (DIM, DIM), F32R, kind="ExternalInput")
    d_fc1T = nc.dram_tensor("fc1T", (DIM, 4 * DIM), F32R, kind="ExternalInput")
    d_fc2T = nc.dram_tensor("fc2T", (4 * DIM, DIM), F32R, kind="ExternalInput")
    d_gamma = nc.dram_tensor("gamma", (DIM,), F32, kind="ExternalInput")
    d_beta = nc.dram_tensor("beta", (DIM,), F32, kind="ExternalInput")
    d_fc1b = nc.dram_tensor("fc1_b", (4 * DIM,), F32, kind="ExternalInput")
    d_fc2b = nc.dram_tensor("fc2_b", (DIM,), F32, kind="ExternalInput")
    d_lam = nc.dram_tensor("lam_rep", (128, DIM), F32, kind="ExternalInput")
    d_out = nc.dram_tensor("dout", (QR, DIM), F32, kind="ExternalOutput")

    def bcast_ap(d1, p=128):
        """DRAM 1-D tensor -> (p, n) AP with partition step 0 (row broadcast)."""
        a = d1.ap()
        return bass.AP(tensor=a.tensor, offset=a.offset, ap=[[0, p], a.ap[0]])

    # Q/K head-dim tiling: Qdim (512) split into 96-row projection tiles so
    # every 32-row (head, half) slice starts at partition 0/32/64 (HW limit).
    # block n = 2h+A lives in tile n//3 at offset 32*(n%3).
    MT_W = [96, 96, 96, 96, 96, 32]
    MT_O = [0, 96, 192, 288, 384, 480]

    with tile.TileContext(nc) as tc:
        with (
            tc.tile_pool(name="const", bufs=1) as const,
            tc.tile_pool(name="work", bufs=2) as work,
            tc.tile_pool(name="small", bufs=4) as small,
            tc.tile_pool(name="med", bufs=1) as med,
            tc.tile_pool(name="upool", bufs=3) as upool,
            tc.tile_pool(name="psS", bufs=2, space="PSUM") as psS,
            tc.tile_pool(name="psO", bufs=4, space="PSUM") as psO,
        ):
            # ---- constants (loaded once, used by all iters) ----
            ident = const.tile([128, 128], F32, tag="ident", name="ident")
            make_identity(nc, ident[:])

            wqT = [const.tile([128, 2 * DIM], F32R, tag=f"wqT{c}", name=f"wqT{c}") for c in range(2)]
            wkT = [const.tile([128, 2 * DIM], F32R, tag=f"wkT{c}", name=f"wkT{c}") for c in range(2)]
            wvT = [const.tile([128, DIM], F32R, tag=f"wvT{c}", name=f"wvT{c}") for c in range(2)]
            woT = [const.tile([128, DIM], F32R, tag=f"woT{c}", name=f"woT{c}") for c in range(2)]
            fc1T = [const.tile([128, 4 * DIM], F32R, tag=f"fc1T{c}", name=f"fc1T{c}") for c in range(2)]
            fc2T = [const.tile([128, DIM], F32R, tag=f"fc2T{c}", name=f"fc2T{c}") for c in range(8)]
            for c in range(2):
                nc.sync.dma_start(wqT[c][:], d_wqT[ts(c, 128), :])
                nc.sync.dma_start(wkT[c][:], d_wkT[ts(c, 128), :])
                nc.sync.dma_start(wvT[c][:], d_wvT[ts(c, 128), :])
                nc.sync.dma_start(woT[c][:], d_woT[ts(c, 128), :])
                nc.sync.dma_start(fc1T[c][:], d_fc1T[ts(c, 128), :])
            for c in range(8):
                nc.sync.dma_start(fc2T[c][:], d_fc2T[ts(c, 128), :])

            gamma_bc = const.tile([128, DIM], F32, tag="gamma_bc", name="gamma_bc")
            beta_bc = const.tile([128, DIM], F32, tag="beta_bc", name="beta_bc")
            fc1b_col = const.tile([128, 8], F32, tag="fc1b_col", name="fc1b_col")
            fc2b_bc = const.tile([128, DIM], F32, tag="fc2b_bc", name="fc2b_bc")
            lam_rep = const.tile([128, DIM], F32, tag="lam_rep", name="lam_rep")
            nc.gpsimd.dma_start(out=gamma_bc[:], in_=bcast_ap(d_gamma))
            nc.gpsimd.dma_start(out=beta_bc[:], in_=bcast_ap(d_beta))
            nc.sync.dma_start(
                fc1b_col[:], d_fc1b.ap().rearrange("(c p) -> p c", p=128)
            )
            nc.gpsimd.dma_start(out=fc2b_bc[:], in_=bcast_ap(d_fc2b))
            nc.sync.dma_start(lam_rep[:], d_lam[:, :])

            # ---- persistent per-iter buffers (shared across iters) ----
            drift_s = [const.tile([128, DIM], F32, tag=f"drift{q}", name=f"drift{q}") for q in range(QT_N)]
            xT = [const.tile([128, QR], F32R, tag=f"xT{c}", name=f"xT{c}") for c in range(2)]
            oceanT = [const.tile([128, No], F32R, tag=f"oceanT{c}", name=f"oceanT{c}") for c in range(2)]
            QTm = [const.tile([MT_W[m], QR], F32R, tag=f"QT{m}", name=f"QT{m}") for m in range(6)]
            KTm = [const.tile([MT_W[m], No], F32R, tag=f"KT{m}", name=f"KT{m}") for m in range(6)]
            vp_dt = F32R if ATT_V2 else UV_DT
            Vp = [const.tile([128, H, 66], vp_dt, tag=f"Vp{k}", name=f"Vp{k}") for k in range(KT_N)]
            out_t = None
            ones_bc = None
            ones_col = const.tile([128, H], F32, tag="ones_col", name="ones_col")
            nc.vector.memset(ones_col[:], 1.0)
            if ATT_V2:
                ones_bc = const.tile([1, 32], F32, tag="ones_bc", name="ones_bc")
                nc.vector.memset(ones_bc[:], 1.0)
            else:
                out_t = [const.tile([128, DIM], F32, tag=f"attout{q}", name=f"attout{q}") for q in range(QT_N)]
            drift1 = [const.tile([128, DIM], F32, tag=f"drift1_{q}", name=f"drift1_{q}") for q in range(QT_N)]
            # SBUF reuse: xT is dead after QT build -> reuse for outT and yT;
            # oceanT is dead after KT/V build -> reuse as the 8 hT chunks.
            outT = xT
            yT = xT
            hT = [oceanT[c // 4][:, ts(c % 4, 512)] for c in range(8)]

            skip_scores = "scores" in skip
            skip_exp = "exp" in skip
            skip_uv = "uv" in skip
            ps_const = None
            ut_const = None
            if skip_scores:
                ps_const = psS.tile([128, 2, 512], F32, tag="psS", name="psS")
                nc.vector.memset(ps_const[:], 0.01)
            if skip_exp:
                u_dt = F32R if ATT_V2 else UV_DT
                ut_const = upool.tile([128, 2, 512], u_dt, tag="U", name="U")
                nc.vector.memset(ut_const[:], 1.0)
            if skip_uv and out_t is not None:
                for q in range(QT_N):
                    nc.vector.memset(out_t[q][:], 0.01)

            def pe_transpose(dst, src_tile, col, qidx):
                pt = psS.tile([128, 2, 512], F32, tag="psS", name="psS")[:, 0, 0:128]
                nc.tensor.transpose(pt, src_tile[:, ts(col, 128)], ident[:])
                nc.vector.tensor_copy(out=dst[:, ts(qidx, 128)], in_=pt)

            for _ in range(iters):
                # ---- load drift slice ----
                for q in range(QT_N):
                    nc.sync.dma_start(drift_s[q][:], d_drift[ts(q, 128), :])

                # ---- LN1 -> x, transpose -> xT ----
                x = [work.tile([128, DIM], F32, tag="x", name="x") for _ in range(QT_N)]
                _layernorm(nc, small, x, drift_s, gamma_bc, beta_bc)
                for q in range(QT_N):
                    for c in range(2):
                        pe_transpose(xT[c], x[q], c, q)

                # ---- load oceanT directly (host-pretransposed) ----
                for c in range(2):
                    nc.sync.dma_start(oceanT[c][:], d_oceanT[ts(c, 128), :])

                # ---- QT = Wq @ xT  (6 tiles of (96|32, 512)) ----
                for m in range(6):
                    w = MT_W[m]
                    pp = psS.tile([128, 2, 512], F32, tag="psS", name="psS")
                    for c in range(2):
                        nc.tensor.matmul(
                            pp[:w, 0, :],
                            (wqT[c][:, ds(MT_O[m], w)]),
                            (xT[c][:]),
                            start=(c == 0),
                            stop=(c == 1),
                        )
                    nc.vector.tensor_copy(out=QTm[m][:], in_=pp[:w, 0, :])

                # ---- KT = Wk @ oceanT (6 tiles of (96|32, 2048)) ----
                for m in range(6):
                    w = MT_W[m]
                    for j in range(4):
                        pp = psS.tile([128, 2, 512], F32, tag="psS", name="psS")
                        for c in range(2):
                            nc.tensor.matmul(
                                pp[:w, 0, :],
                                (wkT[c][:, ds(MT_O[m], w)]),
                                (oceanT[c][:, ts(j, 512)]),
                                start=(c == 0),
                                stop=(c == 1),
                            )
                        nc.vector.tensor_copy(
                            out=KTm[m][:, ts(j, 512)], in_=pp[:w, 0, :]
                        )

                # ---- V' tiles: (128, 8 heads, 66) per k-tile ----
                # cols [0:32]=V_h, [32]=1, [33:65]=lam_h*V_h, [65]=1
                for k in range(KT_N):
                    pp = psS.tile([128, 2, 512], F32, tag="psS", name="psS")
                    pv = pp[:, 0, :DIM]
                    for c in range(2):
                        nc.tensor.matmul(
                            pv,
                            (oceanT[c][:, ts(k, 128)]),
                            (wvT[c][:]),
                            start=(c == 0),
                            stop=(c == 1),
                        )
                    pv3 = pp[:, 0, :DIM].rearrange("p (h d) -> p h d", h=H)
                    nc.vector.tensor_copy(out=Vp[k][:, :, 0:32], in_=pv3)
                    nc.vector.tensor_tensor(
                        out=Vp[k][:, :, 33:65],
                        in0=pv3,
                        in1=lam_rep[:].rearrange("p (h d) -> p h d", h=H),
                        op=OP.mult,
                    )
                    nc.vector.tensor_copy(
                        out=Vp[k][:, :, 32:33], in_=ones_col[:, :, None]
                    )
                    nc.vector.tensor_copy(
                        out=Vp[k][:, :, 65:66], in_=ones_col[:, :, None]
                    )

                # ---- attention per head (V2: V' stationary, out^T direct) ----
                if ATT_V2:
                    for h in range(H):
                        po = None
                        if not skip_uv:
                            po = [
                                psO.tile([33, 512], F32, tag="psO", name="psO")
                                for _ in range(2)
                            ]
                        for A in range(2):
                            n = 2 * h + A
                            m, roff = n // 3, 32 * (n % 3)
                            for kb in range(KT_N // 2):
                                ps = ps_const if skip_scores else psS.tile(
                                    [128, 2, 512], F32, tag="psS", name="psS"
                                )
                                if not skip_scores:
                                    for j in range(2):
                                        k = 2 * kb + j
                                        nc.tensor.matmul(
                                            ps[:, j, :],
                                            KTm[m][ds(roff, 32), ts(k, 128)],
                                            QTm[m][ds(roff, 32), :],
                                            start=True,
                                            stop=True,
                                        )
                                ut = ut_const if skip_exp else upool.tile(
                                    [128, 2, 512], F32R, tag="U", name="U"
                                )
                                if not skip_exp:
                                    nc.scalar.activation(
                                        out=ut[:], in_=ps[:], func=AF.Exp, scale=SCALE
                                    )
                                if not skip_uv:
                                    for j in range(2):
                                        k = 2 * kb + j
                                        nc.tensor.matmul(
                                            po[A][:, :],
                                            Vp[k][:, h, ds(33 * A, 33)],
                                            ut[:, j, :],
                                            start=(k == 0),
                                            stop=(k == KT_N - 1),
                                        )
                        if skip_uv:
                            continue
                        # combine into outT rows [32h:32h+32]:
                        # out^T = num1*r1 - num2*r2 (num2 carries lam already)
                        rr = med.tile([1, 2, 512], F32, tag="rr", name="rr")
                        nc.vector.reciprocal(out=rr[:, 0, :], in_=po[0][32:33, :])
                        nc.vector.reciprocal(out=rr[:, 1, :], in_=po[1][32:33, :])
                        pb = psS.tile([128, 2, 512], F32, tag="psS", name="psS")
                        nc.tensor.matmul(
                            pb[0:32, 0, :], ones_bc[:], rr[:, 0, :],
                            start=True, stop=True,
                        )
                        nc.tensor.matmul(
                            pb[0:32, 1, :], ones_bc[:], rr[:, 1, :],
                            start=True, stop=True,
                        )
                        rb = med.tile([32, 2, 512], F32, tag="rb", name="rb")
                        nc.vector.tensor_copy(out=rb[:], in_=pb[0:32, :, :])
                        tmp2 = med.tile([32, 512], F32, tag="t2", name="t2")
                        nc.vector.tensor_tensor(
                            out=tmp2[:], in0=po[1][0:32, :], in1=rb[:, 1, :],
                            op=OP.mult,
                        )
                        outh = med.tile([32, 512], F32R, tag="outh", name="outh")
                        nc.vector.tensor_tensor(
                            out=outh[:], in0=po[0][0:32, :], in1=rb[:, 0, :],
                            op=OP.mult,
                        )
                        nc.vector.tensor_tensor(
                            out=outh[:], in0=outh[:], in1=tmp2[:], op=OP.subtract
                        )
                        # partition-shift into outT rows [32h % 128] via DMA
                        # (DVE ops cannot write across a differing base partition)
                        nc.sync.dma_start(
                            outT[h // 4][ds(32 * (h % 4), 32), :], outh[:]
                        )
                for h in range(H):
                    if ATT_V2:
                        break
                    po = None
                    if not skip_uv:
                        po = psO.tile([128, 4 * 66], F32, tag="psO", name="psO")
                    for A in range(2):
                        n = 2 * h + A
                        m, roff = n // 3, 32 * (n % 3)
                        voff = 33 * A
                        for kb in range(KT_N // 2):  # batches of 2 k-tiles
                            ps = ps_const if skip_scores else psS.tile(
                                [128, 2, 512], F32, tag="psS", name="psS"
                            )
                            if not skip_scores:
                                for j in range(2):
                                    k = 2 * kb + j
                                    nc.tensor.matmul(
                                        ps[:, j, :],
                                        (KTm[m][ds(roff, 32), ts(k, 128)]),
                                        (QTm[m][ds(roff, 32), :]),
                                        start=True,
                                        stop=True,
                                    )
                            ut = ut_const if skip_exp else upool.tile(
                                [128, 2, 512], UV_DT, tag="U", name="U"
                            )
                            if not skip_exp:
                                nc.scalar.activation(
                                    out=ut[:], in_=ps[:], func=AF.Exp, scale=SCALE
                                )
                            for j in range(2):
                                if skip_uv:
                                    break
                                k = 2 * kb + j
                                for q in range(QT_N):
                                    # one accumulation group for the whole bank:
                                    # start zeroes the full 2KB zero-region, so
                                    # only the very first matmul starts and only
                                    # the very last stops.
                                    nc.tensor.matmul(
                                        po[:, ds(66 * q + voff, 33)],
                                        ut[:, j, ts(q, 128)],
                                        Vp[k][:, h, ds(voff, 33)],
                                        start=(A == 0 and k == 0 and q == 0),
                                        stop=(
                                            A == 1
                                            and k == KT_N - 1
                                            and q == QT_N - 1
                                        ),
                                    )
                    # combine: out_h = U1V*r1 - lamU2V*r2
                    for q in range(QT_N):
                        if skip_uv:
                            break
                        r1 = small.tile([128, 1], F32, tag="r1", name="r1")
                        r2 = small.tile([128, 1], F32, tag="r2", name="r2")
                        nc.vector.reciprocal(out=r1[:], in_=po[:, ds(66 * q + 32, 1)])
                        nc.vector.reciprocal(out=r2[:], in_=po[:, ds(66 * q + 65, 1)])
                        t2 = small.tile([128, 32], F32, tag="t2", name="t2")
                        nc.vector.tensor_scalar_mul(
                            out=t2[:], in0=po[:, ds(66 * q + 33, 32)], scalar1=r2[:]
                        )
                        dst = out_t[q][:, ts(h, 32)]
                        nc.vector.tensor_scalar_mul(
                            out=dst, in0=po[:, ds(66 * q, 32)], scalar1=r1[:]
                        )
                        nc.vector.tensor_tensor(
                            out=dst, in0=dst, in1=t2[:], op=OP.subtract
                        )

                # ---- transpose out -> outT (V1 only; V2 wrote outT directly) ----
                if not ATT_V2:
                    for q in range(QT_N):
                        for c in range(2):
                            pe_transpose(outT[c], out_t[q], c, q)

                # ---- drift1 = drift + out @ WoT ----
                for q in range(QT_N):
                    pp = psS.tile([128, 2, 512], F32, tag="psS", name="psS")
                    pw = pp[:, 0, :DIM]
                    for c in range(2):
                        nc.tensor.matmul(
                            pw,
                            (outT[c][:, ts(q, 128)]),
                            (woT[c][:]),
                            start=(c == 0),
                            stop=(c == 1),
                        )
                    nc.vector.tensor_add(out=drift1[q][:], in0=pw, in1=drift_s[q][:])

                # ---- LN2 -> y, transpose -> yT ----
                y = [work.tile([128, DIM], F32, tag="y", name="y") for _ in range(QT_N)]
                _layernorm(nc, small, y, drift1, gamma_bc, beta_bc)
                for q in range(QT_N):
                    for c in range(2):
                        pe_transpose(yT[c], y[q], c, q)

                # ---- fc1 -> h^T directly: h^T = fc1_w @ y^T, fused bias+gelu ----
                for t8 in range(8):
                    pp = psS.tile([128, 2, 512], F32, tag="psS", name="psS")
                    for c in range(2):
                        nc.tensor.matmul(
                            pp[:, 0, :],
                            fc1T[c][:, ts(t8, 128)],
                            yT[c][:],
                            start=(c == 0),
                            stop=(c == 1),
                        )
                    nc.scalar.activation(
                        out=hT[t8][:],
                        in_=pp[:, 0, :],
                        func=AF.Gelu,
                        bias=fc1b_col[:, t8 : t8 + 1],
                        scale=1.0,
                    )

                # ---- fc2 + bias + residual -> dout ----
                for q in range(QT_N):
                    pp = psS.tile([128, 2, 512], F32, tag="psS", name="psS")
                    pf = pp[:, 0, :DIM]
                    for c in range(8):
                        nc.tensor.matmul(
                            pf,
                            (hT[c][:, ts(q, 128)]),
                            (fc2T[c][:]),
                            start=(c == 0),
                            stop=(c == 7),
                        )
                    do = work.tile([128, DIM], F32, tag="dout_t", name="dout_t")
                    nc.vector.tensor_add(out=do[:], in0=pf, in1=drift1[q][:])
                    nc.vector.tensor_add(out=do[:], in0=do[:], in1=fc2b_bc[:])
                    nc.sync.dma_start(d_out[ts(q, 128), :], do[:])


def make_in_maps(inputs):
    """Host-side prep: slice/shard inputs per core, pre-transpose weights,
    compute lam, build broadcast helpers."""
    f = np.float32
    drift = np.ascontiguousarray(np.asarray(inputs["drift"], f))
    ocean = np.ascontiguousarray(np.asarray(inputs["ocean"], f))
    lq1, lk1 = np.asarray(inputs["lq1"], f), np.asarray(inputs["lk1"], f)
    lq2, lk2 = np.asarray(inputs["lq2"], f), np.asarray(inputs["lk2"], f)
    lam = (
        np.exp(np.sum(lq1 * lk1, -1)) - np.exp(np.sum(lq2 * lk2, -1)) + LAMBDA_INIT
    ).astype(f)  # (H,)
    lam_rep = np.tile(np.repeat(lam, HD)[None, :], (128, 1)).astype(f)  # (128, 256)

    shared = {
        "WqT": np.ascontiguousarray(np.asarray(inputs["Wq"], f).T),
        "WkT": np.ascontiguousarray(np.asarray(inputs["Wk"], f).T),
        "WvT": np.ascontiguousarray(np.asarray(inputs["Wv"], f).T),
        "WoT": np.ascontiguousarray(np.asarray(inputs["Wo"], f).T),
        "fc1T": np.ascontiguousarray(np.asarray(inputs["fc1_w"], f).T),
        "fc2T": np.ascontiguousarray(np.asarray(inputs["fc2_w"], f).T),
        "gamma": np.asarray(inputs["gamma"], f),
        "beta": np.asarray(inputs["beta"], f),
        "fc1_b": np.asarray(inputs["fc1_b"], f),
        "fc2_b": np.asarray(inputs["fc2_b"], f),
        "lam_rep": lam_rep,
    }
    oceanT = [np.ascontiguousarray(ocean[b].T) for b in range(B)]
    in_maps = []
    for c in range(NCORES):
        b = c // (NCORES // B)
        q0 = (c % (NCORES // B)) * QR
        m = dict(shared)
        m["drift_s"] = np.ascontiguousarray(drift[b, q0 : q0 + QR])
        m["oceanT_b"] = oceanT[b]
        in_maps.append(m)
    return in_maps


_CACHED_NC = None


def _get_nc():
    global _CACHED_NC
    if _CACHED_NC is None:
        nc = bacc.Bacc(
            "TRN2",
            target_bir_lowering=False,
            debug=False,
            enable_asserts=False,
            num_devices=NCORES,
        )
        build_program(nc, iters=1)
        nc.compile()
        _CACHED_NC = nc
    return _CACHED_NC


def kernel(**inputs):
    nc = _get_nc()
    in_maps = make_in_maps(inputs)
    res = run_bass_kernel_spmd(nc, in_maps, core_ids=list(range(NCORES)))
    parts = [res.results[c]["dout"] for c in range(NCORES)]
    drift_out = np.stack(parts, 0).reshape(B, Nd, DIM)
    ocean_out = np.asarray(inputs["ocean"], np.float32)
    return drift_out, ocean_out
